# revision 11
# baseline (speedup 1.0000x reference)
"""MoE layer (nn_MoELayer_81630148428171) as a Trainium2 Bass kernel on 8 NeuronCores.

Strategy (expert parallelism, per the sharding hint):
  - Router runs on host (jax-cpu, bitwise-identical ops to the reference).
  - Core e owns expert e's weights only (16.8 MB bf16 vs 134 MB for
    all-experts-per-core) and receives the ~2048 tokens routed to expert e,
    pre-gathered feature-major (xT) and zero-padded to a uniform width CNT
    (max over experts) so all 8 cores run one SPMD program.
  - Device work per core is two dense GEMMs over 512-token chunks:
        hidden^T = gelu(w1[e].T @ xT + b1)    (PE, bf16 in / f32 acc)
        y^T      = w2[e].T @ hidden^T + b2
    with 512-wide rhs (one full PSUM bank per matmul), gelu on the scalar
    engine and the bias add on DVE, both hidden under PE time.
  - No transposes, no indirect DMA, no collectives on device: the host
    applies the top-2 combine weights and scatter-adds the two expert
    contributions per token while unsharding (y^T comes back feature-major).
"""

import math
import numpy as np
import ml_dtypes

import concourse.bacc as bacc
import concourse.mybir as mybir
import concourse.tile as tile
from concourse.bass_utils import run_bass_kernel_spmd

# Problem shapes (hardcoded per contract).
B, SEQ, H = 4, 2048, 1024
T = B * SEQ
FF = 4 * H
E = 8
TOP_K = 2
N_CORES = 8
P = 128

TC = 512               # token-chunk width (= one f32 PSUM bank)
W1C = 512              # w1 columns (FF dim) per resident SBUF slab
W2C = 256              # w2 columns (H dim) per streamed SBUF slab
NCH1 = FF // W1C       # 8 slabs
NCH2 = H // W2C        # 4 slabs

BF16 = mybir.dt.bfloat16
F32 = mybir.dt.float32
NP_BF16 = ml_dtypes.bfloat16

_PROGRAM_CACHE: dict[tuple, object] = {}


# ----------------------------------------------------------------------------
# Host-side routing + sharding
# ----------------------------------------------------------------------------

def _route(x_flat, router_w, router_b):
    """Top-2 routing with bitwise-identical math to the jax reference."""
    try:
        import jax
        import jax.numpy as jnp

        cpu = jax.devices("cpu")[0]

        def f(xf, w, b):
            logits = xf @ w + b
            probs = jax.nn.softmax(logits, axis=-1)
            top_values, top_indices = jax.lax.top_k(probs, TOP_K)
            top_values = top_values / jnp.sum(top_values, axis=-1,
                                              keepdims=True)
            return top_values, top_indices

        with jax.default_device(cpu):
            tv, ti = jax.jit(f)(
                jnp.asarray(x_flat), jnp.asarray(router_w),
                jnp.asarray(router_b))
        tv = np.asarray(tv)
        ti = np.asarray(ti)
    except Exception:
        # numpy fallback (f32, same tie-breaking as lax.top_k for distinct
        # values — differences only possible for exact float ties)
        logits = x_flat @ router_w + router_b
        p = np.exp(logits - logits.max(-1, keepdims=True))
        p /= p.sum(-1, keepdims=True)
        ti = np.argsort(-p, axis=-1, kind="stable")[:, :TOP_K]
        tv = np.take_along_axis(p, ti, axis=-1)
        tv = tv / tv.sum(-1, keepdims=True)
    return (
        ti[:, 0].astype(np.int64),
        ti[:, 1].astype(np.int64),
        tv[:, 0].astype(np.float32),
        tv[:, 1].astype(np.float32),
    )


# ----------------------------------------------------------------------------
# Device program
# ----------------------------------------------------------------------------

def build_program(widths):
    """One SPMD program shared by all 8 cores. `widths` are the per-chunk
    token counts (all TC except possibly the last), runtime-derived
    compile-time constants uniform across cores."""
    nchm = len(widths)
    act_fn = mybir.ActivationFunctionType.Gelu

    nc = bacc.Bacc("TRN2", target_bir_lowering=False, debug=False,
                   num_devices=N_CORES)

    # All inputs arrive pre-tiled to SBUF layout (host formats them) so every
    # DMA is a fully contiguous per-partition read.
    xg_d = nc.dram_tensor("xg", [nchm, P, (H // P) * TC], BF16,
                          kind="ExternalInput")
    w1_d = nc.dram_tensor("w1b", [NCH1, P, (H // P) * W1C], BF16,
                          kind="ExternalInput")
    w2_d = nc.dram_tensor("w2b", [NCH2, P, (FF // P) * W2C], BF16,
                          kind="ExternalInput")
    b1_d = nc.dram_tensor("b1f", [P, FF // P], F32, kind="ExternalInput")
    b2_d = nc.dram_tensor("b2f", [P, H // P], F32, kind="ExternalInput")
    out_d = nc.dram_tensor("out", [nchm, P, (H // P) * TC], BF16,
                           kind="ExternalOutput")

    with tile.TileContext(nc) as tc:
        with (
            tc.tile_pool(name="const", bufs=1) as const_pool,
            tc.tile_pool(name="w1", bufs=1) as w1_pool,
            tc.tile_pool(name="w2", bufs=3) as w2_pool,
            tc.tile_pool(name="xg", bufs=2) as xg_pool,
            tc.tile_pool(name="hid", bufs=1) as hid_pool,
            tc.tile_pool(name="yt", bufs=2) as y_pool,
            tc.tile_pool(name="ps1", bufs=3, space="PSUM") as ps1_pool,
            tc.tile_pool(name="ps2", bufs=3, space="PSUM") as ps2_pool,
        ):
            b1_sb = const_pool.tile([P, FF // P], F32)
            nc.sync.dma_start(out=b1_sb[:], in_=b1_d[:])
            b2_sb = const_pool.tile([P, H // P], F32)
            nc.sync.dma_start(out=b2_sb[:], in_=b2_d[:])

            # Prologue: first token chunk, then the resident w1 slabs. The
            # first m-tile only needs w1[0][:, :, :128] and the first k-half
            # of xg0, so those land as separate small DMAs (subtile deps let
            # the first matmul start ~2-3us in instead of ~10us).
            xg_first = xg_pool.tile([P, H // P, TC], BF16, tag="xg")
            xg0_src = xg_d[0].rearrange("p (ko s) -> p ko s", ko=H // P)
            nc.sync.dma_start(out=xg_first[:, :4, :], in_=xg0_src[:, :4, :])
            nc.sync.dma_start(out=xg_first[:, 4:, :], in_=xg0_src[:, 4:, :])
            w1_tiles = []
            for mc in range(NCH1):
                w1t = w1_pool.tile([P, H // P, W1C], BF16, tag=f"w1_{mc}")
                w1_src = w1_d[mc].rearrange("p (ko m) -> p ko m", ko=H // P)
                if mc == 0:
                    nc.sync.dma_start(out=w1t[:, :, :P], in_=w1_src[:, :, :P])
                    nc.sync.dma_start(out=w1t[:, :, P:], in_=w1_src[:, :, P:])
                else:
                    if mc >= 3:
                        # WAW chain: hold the late w1 slabs off the DMA lanes
                        # until the prologue's gating transfers have landed,
                        # so those get full HBM bandwidth.
                        nc.vector.tensor_copy(out=w1t[:, 0, 0:1],
                                              in_=xg_first[:, 0, 0:1])
                    nc.sync.dma_start(out=w1t[:], in_=w1_src)
                w1_tiles.append(w1t)

            for c in range(nchm):
                W = widths[c]
                if c == 0:
                    xg_sb = xg_first
                else:
                    xg_sb = xg_pool.tile([P, H // P, TC], BF16, tag="xg")
                    nc.sync.dma_start(
                        out=xg_sb[:],
                        in_=xg_d[c].rearrange("p (ko s) -> p ko s",
                                              ko=H // P))

                # ---- mm1: hidden^T = gelu(w1.T @ xT + b1) ----
                hid = hid_pool.tile([P, FF // P, TC], BF16, tag="hid")
                for mc in range(NCH1):
                    for mi in range(W1C // P):
                        m = mc * (W1C // P) + mi
                        ps = ps1_pool.tile([P, TC], F32, tag="ps1")
                        for k in range(H // P):
                            nc.tensor.matmul(
                                ps[:, :W],
                                lhsT=w1_tiles[mc][:, k, mi * P:(mi + 1) * P],
                                rhs=xg_sb[:, k, :W],
                                start=(k == 0),
                                stop=(k == H // P - 1),
                            )
                        nc.scalar.activation(
                            hid[:, m, :W], ps[:, :W], act_fn,
                            bias=b1_sb[:, m:m + 1])

                # ---- mm2: y^T = w2.T @ hidden^T + b2 ----
                y_sb = y_pool.tile([P, H // P, TC], BF16, tag="y")
                for hc in range(NCH2):
                    w2t = w2_pool.tile([P, FF // P, W2C], BF16, tag="w2t")
                    nc.sync.dma_start(
                        out=w2t[:],
                        in_=w2_d[hc].rearrange("p (ko n) -> p ko n",
                                               ko=FF // P))
                    for hi in range(W2C // P):
                        h = hc * (W2C // P) + hi
                        ps = ps2_pool.tile([P, TC], F32, tag="ps2")
                        for k in range(FF // P):
                            nc.tensor.matmul(
                                ps[:, :W],
                                lhsT=w2t[:, k, hi * P:(hi + 1) * P],
                                rhs=hid[:, k, :W],
                                start=(k == 0),
                                stop=(k == FF // P - 1),
                            )
                        # Full-width add: cols >= W read stale-but-finite
                        # PSUM so the output DMA can stay fully contiguous.
                        nc.vector.tensor_scalar_add(
                            y_sb[:, h, :], ps[:, :], b2_sb[:, h:h + 1])
                    # Drain each finished slice of y while mm2 continues so
                    # the kernel tail isn't gated on one 1MB store.
                    h0 = hc * (W2C // P)
                    h1 = h0 + W2C // P
                    nc.sync.dma_start(out=out_d[c, :, h0 * TC:h1 * TC],
                                      in_=y_sb[:, h0:h1, :])

    nc.compile()
    return nc


# ----------------------------------------------------------------------------
# Entry point
# ----------------------------------------------------------------------------

def prepare(x, router_w, router_b, w1, b1, w2, b2):
    """Host-side sharding: returns (nc, in_maps, per-expert combine meta)."""
    x_flat = np.ascontiguousarray(np.asarray(x, np.float32).reshape(T, H))
    e1, e2, c1, c2 = _route(x_flat, np.asarray(router_w), np.asarray(router_b))

    toks, cvs = [], []
    for e in range(E):
        m1 = e1 == e
        m2 = e2 == e
        toks.append(np.concatenate([np.nonzero(m1)[0], np.nonzero(m2)[0]]))
        cvs.append(np.concatenate([c1[m1], c2[m2]]))
    cnts = [len(t) for t in toks]
    cnt_max = max(max(cnts), 1)
    nchm = math.ceil(cnt_max / TC)
    wlast = cnt_max - (nchm - 1) * TC
    widths = (TC,) * (nchm - 1) + (min(TC, (wlast + 1) // 2 * 2),)
    cntp = nchm * TC

    x_flat_bf = x_flat.astype(NP_BF16)
    w1np = np.asarray(w1, np.float32).astype(NP_BF16)
    w2np = np.asarray(w2, np.float32).astype(NP_BF16)
    b1np = np.asarray(b1, np.float32)
    b2np = np.asarray(b2, np.float32)

    in_maps = []
    meta = []
    for e in range(E):
        xg = np.zeros((H, cntp), NP_BF16)
        xg[:, :cnts[e]] = x_flat_bf[toks[e]].T
        xg = np.ascontiguousarray(
            xg.reshape(H // P, P, nchm, TC).transpose(2, 1, 0, 3)
            .reshape(nchm, P, (H // P) * TC))
        w1b = np.ascontiguousarray(
            w1np[e].reshape(H // P, P, NCH1, W1C).transpose(2, 1, 0, 3)
            .reshape(NCH1, P, (H // P) * W1C))
        w2b = np.ascontiguousarray(
            w2np[e].reshape(FF // P, P, NCH2, W2C).transpose(2, 1, 0, 3)
            .reshape(NCH2, P, (FF // P) * W2C))
        b1f = np.ascontiguousarray(b1np[e].reshape(FF // P, P).T)
        b2f = np.ascontiguousarray(b2np[e].reshape(H // P, P).T)
        in_maps.append(dict(xg=xg, w1b=w1b, w2b=w2b, b1f=b1f, b2f=b2f))
        meta.append((toks[e], cvs[e], cnts[e], nchm))

    if widths not in _PROGRAM_CACHE:
        _PROGRAM_CACHE[widths] = build_program(widths)
    return _PROGRAM_CACHE[widths], in_maps, meta


def kernel(x, router_w, router_b, w1, b1, w2, b2):
    nc, in_maps, meta = prepare(x, router_w, router_b, w1, b1, w2, b2)
    res = run_bass_kernel_spmd(nc, in_maps, core_ids=list(range(N_CORES)))
    out_full = np.zeros((T, H), np.float32)
    for e in range(E):
        tk, cv, cnt, nchm = meta[e]
        arr = np.asarray(res.results[e]["out"])
        y = (arr.reshape(nchm, P, H // P, TC).transpose(0, 3, 2, 1)
             .reshape(nchm * TC, H)[:cnt].astype(np.float32))
        out_full[tk] += cv[:, None] * y
    return out_full.reshape(B, SEQ, H)


# revision 16
# speedup vs baseline: 1.0865x; 1.0865x over previous
"""MoE layer (nn_MoELayer_81630148428171) as a Trainium2 Bass kernel on 8 NeuronCores.

Strategy (expert parallelism, per the sharding hint):
  - Router runs on host (jax-cpu, bitwise-identical ops to the reference).
  - Core e owns expert e's weights only (16.8 MB bf16 vs 134 MB for
    all-experts-per-core) and receives the ~2048 tokens routed to expert e,
    pre-gathered feature-major (xT) and zero-padded to a uniform width CNT
    (max over experts) so all 8 cores run one SPMD program.
  - Device work per core is two dense GEMMs over 512-token chunks:
        hidden^T = gelu(w1[e].T @ xT + b1)    (PE, bf16 in / f32 acc)
        y^T      = w2[e].T @ hidden^T + b2
    with 512-wide rhs (one full PSUM bank per matmul), gelu on the scalar
    engine and the bias add on DVE, both hidden under PE time.
  - No transposes, no indirect DMA, no collectives on device: the host
    applies the top-2 combine weights and scatter-adds the two expert
    contributions per token while unsharding (y^T comes back feature-major).
"""

import math
import numpy as np
import ml_dtypes

import concourse.bacc as bacc
import concourse.mybir as mybir
import concourse.tile as tile
from concourse.bass_utils import run_bass_kernel_spmd

# Problem shapes (hardcoded per contract).
B, SEQ, H = 4, 2048, 1024
T = B * SEQ
FF = 4 * H
E = 8
TOP_K = 2
N_CORES = 8
P = 128

TC = 512               # token-chunk width (= one f32 PSUM bank)
W1C = 512              # w1 columns (FF dim) per resident SBUF slab
W2C = 256              # w2 columns (H dim) per streamed SBUF slab
NCH1 = FF // W1C       # 8 slabs
NCH2 = H // W2C        # 4 slabs

BF16 = mybir.dt.bfloat16
F32 = mybir.dt.float32
NP_BF16 = ml_dtypes.bfloat16

_PROGRAM_CACHE: dict[tuple, object] = {}


# ----------------------------------------------------------------------------
# Host-side routing + sharding
# ----------------------------------------------------------------------------

def _route(x_flat, router_w, router_b):
    """Top-2 routing with bitwise-identical math to the jax reference."""
    try:
        import jax
        import jax.numpy as jnp

        cpu = jax.devices("cpu")[0]

        def f(xf, w, b):
            logits = xf @ w + b
            probs = jax.nn.softmax(logits, axis=-1)
            top_values, top_indices = jax.lax.top_k(probs, TOP_K)
            top_values = top_values / jnp.sum(top_values, axis=-1,
                                              keepdims=True)
            return top_values, top_indices

        with jax.default_device(cpu):
            tv, ti = jax.jit(f)(
                jnp.asarray(x_flat), jnp.asarray(router_w),
                jnp.asarray(router_b))
        tv = np.asarray(tv)
        ti = np.asarray(ti)
    except Exception:
        # numpy fallback (f32, same tie-breaking as lax.top_k for distinct
        # values — differences only possible for exact float ties)
        logits = x_flat @ router_w + router_b
        p = np.exp(logits - logits.max(-1, keepdims=True))
        p /= p.sum(-1, keepdims=True)
        ti = np.argsort(-p, axis=-1, kind="stable")[:, :TOP_K]
        tv = np.take_along_axis(p, ti, axis=-1)
        tv = tv / tv.sum(-1, keepdims=True)
    return (
        ti[:, 0].astype(np.int64),
        ti[:, 1].astype(np.int64),
        tv[:, 0].astype(np.float32),
        tv[:, 1].astype(np.float32),
    )


# ----------------------------------------------------------------------------
# Device program
# ----------------------------------------------------------------------------

def build_program(widths):
    """One SPMD program shared by all 8 cores. `widths` are the per-chunk
    token counts (all TC except possibly the last), runtime-derived
    compile-time constants uniform across cores."""
    nchm = len(widths)
    act_fn = mybir.ActivationFunctionType.Gelu

    nc = bacc.Bacc("TRN2", target_bir_lowering=False, debug=False,
                   num_devices=N_CORES)

    # All inputs arrive pre-tiled to SBUF layout (host formats them) so every
    # DMA is a fully contiguous per-partition read.
    xg_d = nc.dram_tensor("xg", [nchm, P, (H // P) * TC], BF16,
                          kind="ExternalInput")
    w1_d = nc.dram_tensor("w1b", [NCH1, P, (H // P) * W1C], BF16,
                          kind="ExternalInput")
    w2_d = nc.dram_tensor("w2b", [NCH2, P, (FF // P) * W2C], BF16,
                          kind="ExternalInput")
    b1_d = nc.dram_tensor("b1f", [P, FF // P], F32, kind="ExternalInput")
    b2_d = nc.dram_tensor("b2f", [P, H // P], F32, kind="ExternalInput")
    out_d = nc.dram_tensor("out", [nchm, P, (H // P) * TC], BF16,
                           kind="ExternalOutput")

    with tile.TileContext(nc) as tc:
        with (
            tc.tile_pool(name="const", bufs=1) as const_pool,
            tc.tile_pool(name="w1", bufs=1) as w1_pool,
            tc.tile_pool(name="w2", bufs=3) as w2_pool,
            tc.tile_pool(name="xg", bufs=2) as xg_pool,
            tc.tile_pool(name="hid", bufs=1) as hid_pool,
            tc.tile_pool(name="yt", bufs=2) as y_pool,
            tc.tile_pool(name="ps1", bufs=3, space="PSUM") as ps1_pool,
            tc.tile_pool(name="ps2", bufs=3, space="PSUM") as ps2_pool,
        ):
            # Prologue. The SP sequencer issues DMAs serially (~650ns each)
            # and its HWDGE ring drains them in order, so the two transfers
            # that gate the first matmul (first k-half of xg0, first m-block
            # of w1[0]) go FIRST on the sync ring; second-priority halves go
            # on the scalar engine's HWDGE ring (drains in parallel), and the
            # bias constants (first needed by gelu at ~6us; SWDGE lands them
            # in ~3us) take the gpsimd path. Subtile deps let the first
            # matmul start ~3us in.
            xg_first = xg_pool.tile([P, H // P, TC], BF16, tag="xg")
            xg0_src = xg_d[0].rearrange("p (ko s) -> p ko s", ko=H // P)
            nc.sync.dma_start(out=xg_first[:, :4, :], in_=xg0_src[:, :4, :])
            w1_tiles = []
            for mc in range(NCH1):
                w1t = w1_pool.tile([P, H // P, W1C], BF16, tag=f"w1_{mc}")
                w1_src = w1_d[mc].rearrange("p (ko m) -> p ko m", ko=H // P)
                if mc == 0:
                    nc.scalar.dma_start(out=w1t[:, :, :P], in_=w1_src[:, :, :P])
                    nc.scalar.dma_start(out=w1t[:, :, P:], in_=w1_src[:, :, P:])
                    nc.sync.dma_start(out=xg_first[:, 4:, :],
                                      in_=xg0_src[:, 4:, :])
                else:
                    nc.sync.dma_start(out=w1t[:], in_=w1_src)
                w1_tiles.append(w1t)
            b1_sb = const_pool.tile([P, FF // P], F32)
            nc.gpsimd.dma_start(out=b1_sb[:], in_=b1_d[:])
            b2_sb = const_pool.tile([P, H // P], F32)
            nc.gpsimd.dma_start(out=b2_sb[:], in_=b2_d[:])

            for c in range(nchm):
                W = widths[c]
                if c == 0:
                    xg_sb = xg_first
                else:
                    xg_sb = xg_pool.tile([P, H // P, TC], BF16, tag="xg")
                    nc.scalar.dma_start(
                        out=xg_sb[:],
                        in_=xg_d[c].rearrange("p (ko s) -> p ko s",
                                              ko=H // P))

                # ---- mm1: hidden^T = gelu(w1.T @ xT + b1) ----
                hid = hid_pool.tile([P, FF // P, TC], BF16, tag="hid")
                for mc in range(NCH1):
                    for mi in range(W1C // P):
                        m = mc * (W1C // P) + mi
                        ps = ps1_pool.tile([P, TC], F32, tag="ps1")
                        for k in range(H // P):
                            nc.tensor.matmul(
                                ps[:, :W],
                                lhsT=w1_tiles[mc][:, k, mi * P:(mi + 1) * P],
                                rhs=xg_sb[:, k, :W],
                                start=(k == 0),
                                stop=(k == H // P - 1),
                            )
                        nc.scalar.activation(
                            hid[:, m, :W], ps[:, :W], act_fn,
                            bias=b1_sb[:, m:m + 1])

                # ---- mm2: y^T = w2.T @ hidden^T + b2 ----
                y_sb = y_pool.tile([P, H // P, TC], BF16, tag="y")
                for hc in range(NCH2):
                    w2t = w2_pool.tile([P, FF // P, W2C], BF16, tag="w2t")
                    nc.sync.dma_start(
                        out=w2t[:],
                        in_=w2_d[hc].rearrange("p (ko n) -> p ko n",
                                               ko=FF // P))
                    for hi in range(W2C // P):
                        h = hc * (W2C // P) + hi
                        ps = ps2_pool.tile([P, TC], F32, tag="ps2")
                        for k in range(FF // P):
                            nc.tensor.matmul(
                                ps[:, :W],
                                lhsT=w2t[:, k, hi * P:(hi + 1) * P],
                                rhs=hid[:, k, :W],
                                start=(k == 0),
                                stop=(k == FF // P - 1),
                            )
                        # Full-width add: cols >= W read stale-but-finite
                        # PSUM so the output DMA can stay fully contiguous.
                        nc.vector.tensor_scalar_add(
                            y_sb[:, h, :], ps[:, :], b2_sb[:, h:h + 1])
                    # Drain each finished slice of y while mm2 continues so
                    # the kernel tail isn't gated on one 1MB store; the very
                    # last chunk drains per h-tile to minimize the tail.
                    h0 = hc * (W2C // P)
                    h1 = h0 + W2C // P
                    if c == nchm - 1 and hc == NCH2 - 1:
                        for hx in range(h0, h1):
                            nc.sync.dma_start(
                                out=out_d[c, :, hx * TC:(hx + 1) * TC],
                                in_=y_sb[:, hx:hx + 1, :])
                    else:
                        nc.sync.dma_start(out=out_d[c, :, h0 * TC:h1 * TC],
                                          in_=y_sb[:, h0:h1, :])

    nc.compile()
    return nc


# ----------------------------------------------------------------------------
# Entry point
# ----------------------------------------------------------------------------

def prepare(x, router_w, router_b, w1, b1, w2, b2):
    """Host-side sharding: returns (nc, in_maps, per-expert combine meta)."""
    x_flat = np.ascontiguousarray(np.asarray(x, np.float32).reshape(T, H))
    e1, e2, c1, c2 = _route(x_flat, np.asarray(router_w), np.asarray(router_b))

    toks, cvs = [], []
    for e in range(E):
        m1 = e1 == e
        m2 = e2 == e
        toks.append(np.concatenate([np.nonzero(m1)[0], np.nonzero(m2)[0]]))
        cvs.append(np.concatenate([c1[m1], c2[m2]]))
    cnts = [len(t) for t in toks]
    cnt_max = max(max(cnts), 1)
    nchm = math.ceil(cnt_max / TC)
    wlast = cnt_max - (nchm - 1) * TC
    widths = (TC,) * (nchm - 1) + (min(TC, (wlast + 1) // 2 * 2),)
    cntp = nchm * TC

    x_flat_bf = x_flat.astype(NP_BF16)
    w1np = np.asarray(w1, np.float32).astype(NP_BF16)
    w2np = np.asarray(w2, np.float32).astype(NP_BF16)
    b1np = np.asarray(b1, np.float32)
    b2np = np.asarray(b2, np.float32)

    in_maps = []
    meta = []
    for e in range(E):
        xg = np.zeros((H, cntp), NP_BF16)
        xg[:, :cnts[e]] = x_flat_bf[toks[e]].T
        xg = np.ascontiguousarray(
            xg.reshape(H // P, P, nchm, TC).transpose(2, 1, 0, 3)
            .reshape(nchm, P, (H // P) * TC))
        w1b = np.ascontiguousarray(
            w1np[e].reshape(H // P, P, NCH1, W1C).transpose(2, 1, 0, 3)
            .reshape(NCH1, P, (H // P) * W1C))
        w2b = np.ascontiguousarray(
            w2np[e].reshape(FF // P, P, NCH2, W2C).transpose(2, 1, 0, 3)
            .reshape(NCH2, P, (FF // P) * W2C))
        b1f = np.ascontiguousarray(b1np[e].reshape(FF // P, P).T)
        b2f = np.ascontiguousarray(b2np[e].reshape(H // P, P).T)
        in_maps.append(dict(xg=xg, w1b=w1b, w2b=w2b, b1f=b1f, b2f=b2f))
        meta.append((toks[e], cvs[e], cnts[e], nchm))

    if widths not in _PROGRAM_CACHE:
        _PROGRAM_CACHE[widths] = build_program(widths)
    return _PROGRAM_CACHE[widths], in_maps, meta


def kernel(x, router_w, router_b, w1, b1, w2, b2):
    nc, in_maps, meta = prepare(x, router_w, router_b, w1, b1, w2, b2)
    res = run_bass_kernel_spmd(nc, in_maps, core_ids=list(range(N_CORES)))
    out_full = np.zeros((T, H), np.float32)
    for e in range(E):
        tk, cv, cnt, nchm = meta[e]
        arr = np.asarray(res.results[e]["out"])
        y = (arr.reshape(nchm, P, H // P, TC).transpose(0, 3, 2, 1)
             .reshape(nchm * TC, H)[:cnt].astype(np.float32))
        out_full[tk] += cv[:, None] * y
    return out_full.reshape(B, SEQ, H)


# revision 17
# speedup vs baseline: 1.3765x; 1.2669x over previous
"""MoE layer (nn_MoELayer_81630148428171) as a Trainium2 Bass kernel on 8 NeuronCores.

Strategy (expert parallelism, per the sharding hint):
  - Router runs on host (jax-cpu, bitwise-identical ops to the reference).
  - Core e owns expert e's weights only (16.8 MB bf16 vs 134 MB for
    all-experts-per-core) and receives the ~2048 tokens routed to expert e,
    pre-gathered feature-major (xT) and zero-padded to a uniform width CNT
    (max over experts) so all 8 cores run one SPMD program.
  - Device work per core is two dense GEMMs over 512-token chunks:
        hidden^T = gelu(w1[e].T @ xT + b1)    (PE, bf16 in / f32 acc)
        y^T      = w2[e].T @ hidden^T + b2
    with 512-wide rhs (one full PSUM bank per matmul), gelu on the scalar
    engine and the bias add on DVE, both hidden under PE time.
  - No transposes, no indirect DMA, no collectives on device: the host
    applies the top-2 combine weights and scatter-adds the two expert
    contributions per token while unsharding (y^T comes back feature-major).
"""

import math
import numpy as np
import ml_dtypes

import concourse.bacc as bacc
import concourse.mybir as mybir
import concourse.tile as tile
from concourse.bass_utils import run_bass_kernel_spmd

# Problem shapes (hardcoded per contract).
B, SEQ, H = 4, 2048, 1024
T = B * SEQ
FF = 4 * H
E = 8
TOP_K = 2
N_CORES = 8
P = 128

TC = 512               # token-chunk width (= one f32 PSUM bank)
W1C = 512              # w1 columns (FF dim) per resident SBUF slab
W2C = 256              # w2 columns (H dim) per streamed SBUF slab
NCH1 = FF // W1C       # 8 slabs
NCH2 = H // W2C        # 4 slabs

BF16 = mybir.dt.bfloat16
F32 = mybir.dt.float32
NP_BF16 = ml_dtypes.bfloat16

_PROGRAM_CACHE: dict[tuple, object] = {}


# ----------------------------------------------------------------------------
# Host-side routing + sharding
# ----------------------------------------------------------------------------

def _route(x_flat, router_w, router_b):
    """Top-2 routing with bitwise-identical math to the jax reference."""
    try:
        import jax
        import jax.numpy as jnp

        cpu = jax.devices("cpu")[0]

        def f(xf, w, b):
            logits = xf @ w + b
            probs = jax.nn.softmax(logits, axis=-1)
            top_values, top_indices = jax.lax.top_k(probs, TOP_K)
            top_values = top_values / jnp.sum(top_values, axis=-1,
                                              keepdims=True)
            return top_values, top_indices

        with jax.default_device(cpu):
            tv, ti = jax.jit(f)(
                jnp.asarray(x_flat), jnp.asarray(router_w),
                jnp.asarray(router_b))
        tv = np.asarray(tv)
        ti = np.asarray(ti)
    except Exception:
        # numpy fallback (f32, same tie-breaking as lax.top_k for distinct
        # values — differences only possible for exact float ties)
        logits = x_flat @ router_w + router_b
        p = np.exp(logits - logits.max(-1, keepdims=True))
        p /= p.sum(-1, keepdims=True)
        ti = np.argsort(-p, axis=-1, kind="stable")[:, :TOP_K]
        tv = np.take_along_axis(p, ti, axis=-1)
        tv = tv / tv.sum(-1, keepdims=True)
    return (
        ti[:, 0].astype(np.int64),
        ti[:, 1].astype(np.int64),
        tv[:, 0].astype(np.float32),
        tv[:, 1].astype(np.float32),
    )


# ----------------------------------------------------------------------------
# Device program
# ----------------------------------------------------------------------------

def build_program(widths):
    """One SPMD program shared by all 8 cores. `widths` are the per-chunk
    token counts (all TC except possibly the last), runtime-derived
    compile-time constants uniform across cores."""
    nchm = len(widths)
    act_fn = mybir.ActivationFunctionType.Gelu

    nc = bacc.Bacc("TRN2", target_bir_lowering=False, debug=False,
                   num_devices=N_CORES)

    # All inputs arrive pre-tiled to SBUF layout (host formats them) so every
    # DMA is a fully contiguous per-partition read.
    xg_d = nc.dram_tensor("xg", [nchm, P, (H // P) * TC], BF16,
                          kind="ExternalInput")
    w1_d = nc.dram_tensor("w1b", [NCH1, P, (H // P) * W1C], BF16,
                          kind="ExternalInput")
    w2_d = nc.dram_tensor("w2b", [NCH2, P, (FF // P) * W2C], BF16,
                          kind="ExternalInput")
    b1_d = nc.dram_tensor("b1f", [P, FF // P], F32, kind="ExternalInput")
    b2_d = nc.dram_tensor("b2f", [P, H // P], F32, kind="ExternalInput")
    out_d = nc.dram_tensor("out", [nchm, P, (H // P) * TC], BF16,
                           kind="ExternalOutput")

    with tile.TileContext(nc) as tc:
        with (
            tc.tile_pool(name="const", bufs=1) as const_pool,
            tc.tile_pool(name="w1", bufs=1) as w1_pool,
            tc.tile_pool(name="w2", bufs=3) as w2_pool,
            tc.tile_pool(name="xg", bufs=2) as xg_pool,
            tc.tile_pool(name="hid", bufs=1) as hid_pool,
            tc.tile_pool(name="yt", bufs=2) as y_pool,
            tc.tile_pool(name="ps1", bufs=3, space="PSUM") as ps1_pool,
            tc.tile_pool(name="ps2", bufs=3, space="PSUM") as ps2_pool,
        ):
            # Prologue. The SP sequencer issues DMAs serially (~650ns each)
            # and its HWDGE ring drains them in order, so the two transfers
            # that gate the first matmul (first k-half of xg0, first m-block
            # of w1[0]) go FIRST on the sync ring; second-priority halves go
            # on the scalar engine's HWDGE ring (drains in parallel), and the
            # bias constants (first needed by gelu at ~6us; SWDGE lands them
            # in ~3us) take the gpsimd path. Subtile deps let the first
            # matmul start ~3us in.
            xg_first = xg_pool.tile([P, H // P, TC], BF16, tag="xg")
            xg0_src = xg_d[0].rearrange("p (ko s) -> p ko s", ko=H // P)
            nc.sync.dma_start(out=xg_first[:, :1, :], in_=xg0_src[:, :1, :])
            nc.sync.dma_start(out=xg_first[:, 1:4, :], in_=xg0_src[:, 1:4, :])
            w1_tiles = []
            for mc in range(NCH1):
                w1t = w1_pool.tile([P, H // P, W1C], BF16, tag=f"w1_{mc}")
                w1_src = w1_d[mc].rearrange("p (ko m) -> p ko m", ko=H // P)
                if mc == 0:
                    nc.scalar.dma_start(out=w1t[:, :1, :P],
                                        in_=w1_src[:, :1, :P])
                    nc.scalar.dma_start(out=w1t[:, 1:, :P],
                                        in_=w1_src[:, 1:, :P])
                    nc.scalar.dma_start(out=w1t[:, :, P:], in_=w1_src[:, :, P:])
                    nc.sync.dma_start(out=xg_first[:, 4:, :],
                                      in_=xg0_src[:, 4:, :])
                else:
                    nc.sync.dma_start(out=w1t[:], in_=w1_src)
                w1_tiles.append(w1t)
            b1_sb = const_pool.tile([P, FF // P], F32)
            nc.gpsimd.dma_start(out=b1_sb[:], in_=b1_d[:])
            b2_sb = const_pool.tile([P, H // P], F32)
            nc.gpsimd.dma_start(out=b2_sb[:], in_=b2_d[:])

            for c in range(nchm):
                W = widths[c]
                if c == 0:
                    xg_sb = xg_first
                else:
                    xg_sb = xg_pool.tile([P, H // P, TC], BF16, tag="xg")
                    nc.scalar.dma_start(
                        out=xg_sb[:],
                        in_=xg_d[c].rearrange("p (ko s) -> p ko s",
                                              ko=H // P))

                # ---- mm1: hidden^T = gelu(w1.T @ xT + b1) ----
                hid = hid_pool.tile([P, FF // P, TC], BF16, tag="hid")
                for mc in range(NCH1):
                    for mi in range(W1C // P):
                        m = mc * (W1C // P) + mi
                        ps = ps1_pool.tile([P, TC], F32, tag="ps1")
                        for k in range(H // P):
                            nc.tensor.matmul(
                                ps[:, :W],
                                lhsT=w1_tiles[mc][:, k, mi * P:(mi + 1) * P],
                                rhs=xg_sb[:, k, :W],
                                start=(k == 0),
                                stop=(k == H // P - 1),
                            )
                        nc.scalar.activation(
                            hid[:, m, :W], ps[:, :W], act_fn,
                            bias=b1_sb[:, m:m + 1])

                # ---- mm2: y^T = w2.T @ hidden^T + b2 ----
                y_sb = y_pool.tile([P, H // P, TC], BF16, tag="y")
                for hc in range(NCH2):
                    w2t = w2_pool.tile([P, FF // P, W2C], BF16, tag="w2t")
                    nc.sync.dma_start(
                        out=w2t[:],
                        in_=w2_d[hc].rearrange("p (ko n) -> p ko n",
                                               ko=FF // P))
                    for hi in range(W2C // P):
                        h = hc * (W2C // P) + hi
                        ps = ps2_pool.tile([P, TC], F32, tag="ps2")
                        for k in range(FF // P):
                            nc.tensor.matmul(
                                ps[:, :W],
                                lhsT=w2t[:, k, hi * P:(hi + 1) * P],
                                rhs=hid[:, k, :W],
                                start=(k == 0),
                                stop=(k == FF // P - 1),
                            )
                        # Full-width add: cols >= W read stale-but-finite
                        # PSUM so the output DMA can stay fully contiguous.
                        nc.vector.tensor_scalar_add(
                            y_sb[:, h, :], ps[:, :], b2_sb[:, h:h + 1])
                    # Drain each finished slice of y while mm2 continues so
                    # the kernel tail isn't gated on one 1MB store; the very
                    # last chunk drains per h-tile to minimize the tail.
                    h0 = hc * (W2C // P)
                    h1 = h0 + W2C // P
                    if c == nchm - 1 and hc == NCH2 - 1:
                        for hx in range(h0, h1):
                            nc.sync.dma_start(
                                out=out_d[c, :, hx * TC:(hx + 1) * TC],
                                in_=y_sb[:, hx:hx + 1, :])
                    else:
                        nc.sync.dma_start(out=out_d[c, :, h0 * TC:h1 * TC],
                                          in_=y_sb[:, h0:h1, :])

    nc.compile()
    return nc


# ----------------------------------------------------------------------------
# Entry point
# ----------------------------------------------------------------------------

def prepare(x, router_w, router_b, w1, b1, w2, b2):
    """Host-side sharding: returns (nc, in_maps, per-expert combine meta)."""
    x_flat = np.ascontiguousarray(np.asarray(x, np.float32).reshape(T, H))
    e1, e2, c1, c2 = _route(x_flat, np.asarray(router_w), np.asarray(router_b))

    toks, cvs = [], []
    for e in range(E):
        m1 = e1 == e
        m2 = e2 == e
        toks.append(np.concatenate([np.nonzero(m1)[0], np.nonzero(m2)[0]]))
        cvs.append(np.concatenate([c1[m1], c2[m2]]))
    cnts = [len(t) for t in toks]
    cnt_max = max(max(cnts), 1)
    nchm = math.ceil(cnt_max / TC)
    wlast = cnt_max - (nchm - 1) * TC
    widths = (TC,) * (nchm - 1) + (min(TC, (wlast + 1) // 2 * 2),)
    cntp = nchm * TC

    x_flat_bf = x_flat.astype(NP_BF16)
    w1np = np.asarray(w1, np.float32).astype(NP_BF16)
    w2np = np.asarray(w2, np.float32).astype(NP_BF16)
    b1np = np.asarray(b1, np.float32)
    b2np = np.asarray(b2, np.float32)

    in_maps = []
    meta = []
    for e in range(E):
        xg = np.zeros((H, cntp), NP_BF16)
        xg[:, :cnts[e]] = x_flat_bf[toks[e]].T
        xg = np.ascontiguousarray(
            xg.reshape(H // P, P, nchm, TC).transpose(2, 1, 0, 3)
            .reshape(nchm, P, (H // P) * TC))
        w1b = np.ascontiguousarray(
            w1np[e].reshape(H // P, P, NCH1, W1C).transpose(2, 1, 0, 3)
            .reshape(NCH1, P, (H // P) * W1C))
        w2b = np.ascontiguousarray(
            w2np[e].reshape(FF // P, P, NCH2, W2C).transpose(2, 1, 0, 3)
            .reshape(NCH2, P, (FF // P) * W2C))
        b1f = np.ascontiguousarray(b1np[e].reshape(FF // P, P).T)
        b2f = np.ascontiguousarray(b2np[e].reshape(H // P, P).T)
        in_maps.append(dict(xg=xg, w1b=w1b, w2b=w2b, b1f=b1f, b2f=b2f))
        meta.append((toks[e], cvs[e], cnts[e], nchm))

    if widths not in _PROGRAM_CACHE:
        _PROGRAM_CACHE[widths] = build_program(widths)
    return _PROGRAM_CACHE[widths], in_maps, meta


def kernel(x, router_w, router_b, w1, b1, w2, b2):
    nc, in_maps, meta = prepare(x, router_w, router_b, w1, b1, w2, b2)
    res = run_bass_kernel_spmd(nc, in_maps, core_ids=list(range(N_CORES)))
    out_full = np.zeros((T, H), np.float32)
    for e in range(E):
        tk, cv, cnt, nchm = meta[e]
        arr = np.asarray(res.results[e]["out"])
        y = (arr.reshape(nchm, P, H // P, TC).transpose(0, 3, 2, 1)
             .reshape(nchm * TC, H)[:cnt].astype(np.float32))
        out_full[tk] += cv[:, None] * y
    return out_full.reshape(B, SEQ, H)
